# revision 19
# baseline (speedup 1.0000x reference)
"""FBank preprocessor (kaldi-style log-mel) as a Bass/Trainium2 kernel.

Pipeline per 1-sec waveform (48 kHz):
  frame (98 x 1200, hop 480) -> remove DC -> preemphasis 0.97 -> hann
  -> zero-pad 2048 -> |rfft|^2 -> mel (128 banks) -> log -> pad 98->128
  -> (x - MEAN) / (2*STD)

Everything up to the power spectrum is linear in the frame samples, so
DC-removal/preemphasis/hann/rDFT are folded into two dense (1280 x 1024)
cos/sin matrices on the host.  On-device per 5-waveform block:
  - one strided DMA load of raw frames (t on partitions, samples on free)
  - PE transposes to get samples on partitions (moving operands)
  - f32r matmuls against the folded DFT matrices (contract n, 10 chunks)
  - ACT Square PSUM->SBUF, mel matmul over stacked [Re^2; Im^2]
  - clamp/log/scale epilogue, PE transpose back, DMA out

Data parallel over 8 NeuronCores: 64 waveforms each.
"""

import functools

import numpy as np

import concourse.bass as bass
import concourse.bacc as bacc
import concourse.tile as tile
from concourse import mybir
from concourse import bass_utils

F32 = mybir.dt.float32
F32R = mybir.dt.float32r

SR = 48000
WIN = 1200
HOP = 480
PADWIN = 2048
NMEL = 128
TFRAMES = 128
NFRAMES = 98
PREEMPH = 0.97
MEAN = -4.2677393
STD = 4.5689974
EPS = 1.1920928955078125e-07

NCORES = 8
B = 512
BPC = B // NCORES          # 64 waveforms per core
NCHUNK = 10                # contract chunks of 128 samples (1280 >= 1200)
KBINS = 1024               # rfft bins 0..1023 (bin 1024 has zero mel weight)
NW = 5                     # waveforms per block (N = 5*98 = 490 <= 512)

EPS_S = float(EPS * np.exp(-MEAN))
OUT_SCALE = float(1.0 / (2.0 * STD))
PADV = float((0.0 - MEAN) / (2.0 * STD))
AM = 8.0                   # fp8 DFT matrix pre-scale (folded out of mel fb)
NFPAD = 512                # fp8 moving tile stride (DoubleRow needs %16 == 0)


def _mel_banks_f64():
    # torchaudio.compliance.kaldi.get_mel_banks (low 20 Hz, high nyquist)
    fft_bin_width = SR / PADWIN
    mel = lambda f: 1127.0 * np.log(1.0 + f / 700.0)
    mel_low, mel_high = mel(20.0), mel(SR / 2.0)
    delta = (mel_high - mel_low) / (NMEL + 1)
    left = mel_low + np.arange(NMEL)[:, None] * delta
    center = left + delta
    right = center + delta
    m = mel(fft_bin_width * np.arange(KBINS))[None, :]
    up = (m - left) / (center - left)
    down = (right - m) / (right - center)
    return np.maximum(0.0, np.minimum(up, down))  # (128, 1024)


def _build_host_constants():
    # T = diag(hann) @ P_preemph @ (I - ones/WIN), all (WIN x WIN), f64
    n = np.arange(WIN)
    hann = 0.5 - 0.5 * np.cos(2.0 * np.pi * n / (WIN - 1))
    T = np.eye(WIN) - np.ones((WIN, WIN)) / WIN
    P = np.eye(WIN)
    P[np.arange(1, WIN), np.arange(WIN - 1)] -= PREEMPH
    P[0, 0] -= PREEMPH            # kaldi replicate pad: first sample pairs itself
    T = P @ T
    T = hann[:, None] * T

    k = np.arange(KBINS)
    ang = 2.0 * np.pi * np.outer(n, k) / PADWIN      # (1200, 1024)
    d_re = T.T @ np.cos(ang)                          # (1200, 1024)
    d_im = T.T @ (-np.sin(ang))

    D = np.zeros((NCHUNK * 128, 2 * KBINS), np.float64)
    D[:WIN, :KBINS] = d_re
    D[:WIN, KBINS:] = d_im
    # (128 partitions = n % 128, NCHUNK, 2048 k-cols)
    dftm = D.reshape(NCHUNK, 128, 2 * KBINS).transpose(1, 0, 2)

    # spectrum comes out scaled by AM; fold exp(-MEAN)/AM^2 into mel fb
    fbs = _mel_banks_f64() * (np.exp(-MEAN) / (AM * AM))
    fbd = np.zeros((128, 8, 128), np.float64)
    for kk in range(8):
        fbd[:, kk, :] = fbs[:, kk * 128:(kk + 1) * 128].T
    import ml_dtypes
    return (
        np.ascontiguousarray(dftm * AM, dtype=ml_dtypes.float8_e4m3),
        np.ascontiguousarray(fbd.astype(np.float32), dtype=ml_dtypes.bfloat16),
    )


def _blocks():
    out = []
    b0 = 0
    while b0 < BPC:
        out.append((b0, min(NW, BPC - b0)))
        b0 += NW
    return out


@functools.lru_cache(maxsize=1)
def _build_nc():
    nc = bacc.Bacc("TRN2", target_bir_lowering=False, debug=False,
                   num_devices=NCORES)

    WAVE = nc.dram_tensor("wave", [BPC, SR], mybir.dt.bfloat16,
                        kind="ExternalInput")
    DFT = nc.dram_tensor("dftm", [128, NCHUNK, 2 * KBINS], mybir.dt.float8e4,
                         kind="ExternalInput")
    FBD = nc.dram_tensor("fbd", [128, 8, 128], mybir.dt.bfloat16,
                         kind="ExternalInput")
    IDT = nc.dram_tensor("ident", [128, 128], mybir.dt.bfloat16,
                        kind="ExternalInput")
    IDTR = nc.dram_tensor("identr", [128, 128], F32R, kind="ExternalInput")
    OUT = nc.dram_tensor("out", [BPC, TFRAMES, NMEL], F32,
                         kind="ExternalOutput")

    def wave_ap(offset, dims):
        return bass.AP(tensor=WAVE, offset=offset, ap=list(dims))

    def out_ap(offset, dims):
        return bass.AP(tensor=OUT, offset=offset, ap=list(dims))

    with tile.TileContext(nc) as tc:
        with tc.tile_pool(name="const", bufs=1) as constp, \
             tc.tile_pool(name="mvb", bufs=10) as mvbp, \
             tc.tile_pool(name="mvt", bufs=10) as mvtp, \
             tc.tile_pool(name="mv", bufs=2) as mvp, \
             tc.tile_pool(name="sq", bufs=14) as sqp, \
             tc.tile_pool(name="epi", bufs=2) as epp, \
             tc.tile_pool(name="dft_ps", bufs=2, space="PSUM") as dftps, \
             tc.tile_pool(name="mel_ps", bufs=2, space="PSUM") as melps, \
             tc.tile_pool(name="otr_ps", bufs=1, space="PSUM") as otrps:

            identb = constp.tile([128, 128], mybir.dt.bfloat16)
            nc.sync.dma_start(out=identb[:], in_=IDT.ap())
            identr = constp.tile([128, 128], F32R)
            nc.scalar.dma_start(out=identr[:], in_=IDTR.ap())

            # frames arrive pre-transposed via HWDGE xbar DMA-transpose:
            # main [96,1280]->[128,10,96] plus a 16-frame tail [82:98) into a
            # separate tile (98 % 16 != 0), both full-tile contiguous dests.
            TMAIN, TTAIL0 = 96, 82
            ld_pending = {}

            def issue_ld(b0, nw):
                tiles = []
                for wb in range(nw):
                    mb = mvbp.tile([128, NCHUNK, TMAIN], mybir.dt.bfloat16,
                                   tag="mvb")
                    nc.sync.dma_start(
                        out=mb[:],
                        in_=wave_ap((b0 + wb) * SR,
                                    [[HOP, TMAIN], [1, NCHUNK * 128]]),
                        transpose=True,
                    )
                    mt = mvtp.tile([128, NCHUNK, 16], mybir.dt.bfloat16,
                                   tag="mvt")
                    nc.sync.dma_start(
                        out=mt[:],
                        in_=wave_ap((b0 + wb) * SR + TTAIL0 * HOP,
                                    [[HOP, 16], [1, NCHUNK * 128]]),
                        transpose=True,
                    )
                    tiles.append((mb, mt))
                ld_pending[b0] = tiles

            blocks = _blocks()
            issue_ld(*blocks[0])

            dftm = constp.tile([128, NCHUNK, 2 * KBINS], mybir.dt.float8e4)
            for half in range(2):
                for c in range(NCHUNK):
                    eng = nc.scalar if c % 2 == 0 else nc.sync
                    eng.dma_start(
                        out=dftm[:, c, half * KBINS:(half + 1) * KBINS],
                        in_=DFT.ap()[:, c, half * KBINS:(half + 1) * KBINS],
                    )
            fbd = constp.tile([128, 8, 128], mybir.dt.bfloat16)
            nc.scalar.dma_start(out=fbd[:], in_=FBD.ap())
            PADG = 8  # waveforms per pad DMA
            padt = constp.tile([TFRAMES - NFRAMES, PADG, NMEL], F32)
            nc.vector.memset(padt[:], PADV)
            for bi, (b0, nw) in enumerate(blocks):
                nf = nw * NFRAMES
                ld = ld_pending.pop(b0)
                if bi + 1 < len(blocks):
                    issue_ld(*blocks[bi + 1])

                # cast the pre-transposed bf16 frames to fp8 moving tiles
                mv = mvp.tile([128, NCHUNK, NFPAD], mybir.dt.float8e4,
                              tag="mv")
                for wb in range(nw):
                    mb, mt = ld[wb]
                    f0 = wb * NFRAMES
                    nc.vector.tensor_copy(
                        mv[:, :, f0:f0 + TMAIN], mb[:])
                    nc.vector.tensor_copy(
                        mv[:, :, f0 + TMAIN:f0 + NFRAMES],
                        mt[:, :, TMAIN - TTAIL0:])

                # DFT (cos/sin folded with preprocessing), power spectrum
                pw = []
                for kk in range(8):
                    halves = []
                    for half in range(2):
                        base = half * KBINS + kk * 128
                        ps = dftps.tile([128, nf], F32, tag="dftps")
                        for c2 in range(NCHUNK // 2):
                            nc.tensor.matmul(
                                ps[:],
                                dftm[:, 2 * c2:2 * c2 + 2, base:base + 128],
                                mv[:, 2 * c2:2 * c2 + 2, :nf],
                                start=(c2 == 0), stop=(c2 == NCHUNK // 2 - 1),
                                perf_mode=mybir.MatmulPerfMode.DoubleRow,
                            )
                        st = sqp.tile([128, nf], mybir.dt.bfloat16, tag="sq")
                        nc.scalar.square(st[:], ps[:])
                        halves.append(st)
                    pt = sqp.tile([128, nf], mybir.dt.bfloat16, tag="pw")
                    nc.vector.tensor_add(pt[:], halves[0][:], halves[1][:])
                    pw.append(pt)

                # mel: contract Re^2+Im^2 (8 chunks of 128 bins)
                mel = melps.tile([128, nf], F32, tag="mel")
                for kk in range(8):
                    nc.tensor.matmul(mel[:], fbd[:, kk, :], pw[kk][:],
                                     start=(kk == 0), stop=(kk == 7))

                # log-mel + normalize: (ln(max(mel', eps')))/(2*std)
                ot = epp.tile([128, nf], F32R, tag="ot")
                nc.vector.tensor_scalar_max(ot[:], mel[:], EPS_S)
                nc.scalar.activation(ot[:], ot[:],
                                     mybir.ActivationFunctionType.Ln)
                nc.vector.tensor_scalar_mul(ot[:], ot[:], OUT_SCALE)

                # transpose back to (frames on partitions, mel on free)
                otr = otrps.tile([NFRAMES, nw * 128], F32R, tag="otr")
                for wb in range(nw):
                    nc.tensor.transpose(
                        otr[:, wb * 128:(wb + 1) * 128],
                        ot[:, wb * NFRAMES:(wb + 1) * NFRAMES],
                        identr[:],
                    )
                oc = epp.tile([NFRAMES, nw, NMEL], F32, tag="oc")
                nc.vector.tensor_copy(oc[:], otr[:].rearrange(
                    "p (w m) -> p w m", w=nw))
                nc.scalar.dma_start(
                    out=out_ap(b0 * TFRAMES * NMEL,
                               [[NMEL, NFRAMES],
                                [TFRAMES * NMEL, nw],
                                [1, NMEL]]),
                    in_=oc[:],
                )

            # constant pad rows (frames 98..127) for every waveform
            for g0 in range(0, BPC, PADG):
                nc.scalar.dma_start(
                    out=out_ap(g0 * TFRAMES * NMEL + NFRAMES * NMEL,
                               [[NMEL, TFRAMES - NFRAMES],
                                [TFRAMES * NMEL, PADG],
                                [1, NMEL]]),
                    in_=padt[:],
                )

    nc.compile()
    return nc


@functools.lru_cache(maxsize=1)
def _host_constants():
    return _build_host_constants()


def make_in_maps(waveform):
    import ml_dtypes
    waveform = np.ascontiguousarray(np.asarray(waveform, dtype=np.float32))
    assert waveform.shape == (B, SR), waveform.shape
    dftm, fbd = _host_constants()
    shards = waveform.reshape(NCORES, BPC, SR).astype(ml_dtypes.bfloat16)
    ident = np.eye(128, dtype=ml_dtypes.bfloat16)
    identr = np.eye(128, dtype=np.float32)
    return [
        {"wave": np.ascontiguousarray(shards[c]), "dftm": dftm, "fbd": fbd,
         "ident": ident, "identr": identr}
        for c in range(NCORES)
    ]


def kernel(waveform):
    nc = _build_nc()
    in_maps = make_in_maps(waveform)
    res = bass_utils.run_bass_kernel_spmd(
        nc, in_maps, core_ids=list(range(NCORES)), trace=False
    )
    return np.concatenate([res.results[c]["out"] for c in range(NCORES)], axis=0)



# revision 23
# speedup vs baseline: 1.3379x; 1.3379x over previous
"""FBank preprocessor (kaldi-style log-mel) as a Bass/Trainium2 kernel.

Pipeline per 1-sec waveform (48 kHz):
  frame (98 x 1200, hop 480) -> remove DC -> preemphasis 0.97 -> hann
  -> zero-pad 2048 -> |rfft|^2 -> mel (128 banks) -> log -> pad 98->128
  -> (x - MEAN) / (2*STD)

Everything up to the power spectrum is linear in the frame samples, so
DC-removal/preemphasis/hann/rDFT are folded into two dense (1280 x 1024)
cos/sin matrices on the host.  On-device per 5-waveform block:
  - one strided DMA load of raw frames (t on partitions, samples on free)
  - PE transposes to get samples on partitions (moving operands)
  - f32r matmuls against the folded DFT matrices (contract n, 10 chunks)
  - ACT Square PSUM->SBUF, mel matmul over stacked [Re^2; Im^2]
  - clamp/log/scale epilogue, PE transpose back, DMA out

Data parallel over 8 NeuronCores: 64 waveforms each.
"""

import functools

import numpy as np

import concourse.bass as bass
import concourse.bacc as bacc
import concourse.tile as tile
from concourse import mybir
from concourse import bass_utils

F32 = mybir.dt.float32
F32R = mybir.dt.float32r

SR = 48000
WIN = 1200
HOP = 480
PADWIN = 2048
NMEL = 128
TFRAMES = 128
NFRAMES = 98
PREEMPH = 0.97
MEAN = -4.2677393
STD = 4.5689974
EPS = 1.1920928955078125e-07

NCORES = 8
B = 512
BPC = B // NCORES          # 64 waveforms per core
NCHUNK = 10                # contract chunks of 128 samples (1280 >= 1200)
KBINS = 1024               # rfft bins 0..1023 (bin 1024 has zero mel weight)
NW = 5                     # waveforms per block (N = 5*98 = 490 <= 512)

EPS_S = float(EPS * np.exp(-MEAN))
OUT_SCALE = float(1.0 / (2.0 * STD))
PADV = float((0.0 - MEAN) / (2.0 * STD))
AM = 8.0                   # fp8 DFT matrix pre-scale (folded out of mel fb)
NFPAD = 512                # fp8 moving tile stride (DoubleRow needs %16 == 0)


def _mel_banks_f64():
    # torchaudio.compliance.kaldi.get_mel_banks (low 20 Hz, high nyquist)
    fft_bin_width = SR / PADWIN
    mel = lambda f: 1127.0 * np.log(1.0 + f / 700.0)
    mel_low, mel_high = mel(20.0), mel(SR / 2.0)
    delta = (mel_high - mel_low) / (NMEL + 1)
    left = mel_low + np.arange(NMEL)[:, None] * delta
    center = left + delta
    right = center + delta
    m = mel(fft_bin_width * np.arange(KBINS))[None, :]
    up = (m - left) / (center - left)
    down = (right - m) / (right - center)
    return np.maximum(0.0, np.minimum(up, down))  # (128, 1024)


def _build_host_constants():
    # T = diag(hann) @ P_preemph @ (I - ones/WIN), all (WIN x WIN), f64
    n = np.arange(WIN)
    hann = 0.5 - 0.5 * np.cos(2.0 * np.pi * n / (WIN - 1))
    T = np.eye(WIN) - np.ones((WIN, WIN)) / WIN
    P = np.eye(WIN)
    P[np.arange(1, WIN), np.arange(WIN - 1)] -= PREEMPH
    P[0, 0] -= PREEMPH            # kaldi replicate pad: first sample pairs itself
    T = P @ T
    T = hann[:, None] * T

    k = np.arange(KBINS)
    ang = 2.0 * np.pi * np.outer(n, k) / PADWIN      # (1200, 1024)
    d_re = T.T @ np.cos(ang)                          # (1200, 1024)
    d_im = T.T @ (-np.sin(ang))

    D = np.zeros((NCHUNK * 128, 2 * KBINS), np.float64)
    D[:WIN, :KBINS] = d_re
    D[:WIN, KBINS:] = d_im
    # (128 partitions = n % 128, NCHUNK, 2048 k-cols)
    dftm = D.reshape(NCHUNK, 128, 2 * KBINS).transpose(1, 0, 2)

    # spectrum comes out scaled by AM; fold exp(-MEAN)/AM^2 into mel fb
    fbs = _mel_banks_f64() * (np.exp(-MEAN) / (AM * AM))
    fbd = np.zeros((128, 8, 128), np.float64)
    for kk in range(8):
        fbd[:, kk, :] = fbs[:, kk * 128:(kk + 1) * 128].T
    import ml_dtypes
    return (
        np.ascontiguousarray(dftm * AM, dtype=ml_dtypes.float8_e4m3),
        np.ascontiguousarray(fbd.astype(np.float32), dtype=ml_dtypes.bfloat16),
    )


def _blocks():
    out = []
    b0 = 0
    while b0 < BPC:
        out.append((b0, min(NW, BPC - b0)))
        b0 += NW
    return out


@functools.lru_cache(maxsize=1)
def _build_nc():
    nc = bacc.Bacc("TRN2", target_bir_lowering=False, debug=False,
                   num_devices=NCORES)

    WAVE = nc.dram_tensor("wave", [BPC, SR], mybir.dt.bfloat16,
                        kind="ExternalInput")
    DFT = nc.dram_tensor("dftm", [128, NCHUNK, 2 * KBINS], mybir.dt.float8e4,
                         kind="ExternalInput")
    FBD = nc.dram_tensor("fbd", [128, 8, 128], mybir.dt.bfloat16,
                         kind="ExternalInput")
    IDT = nc.dram_tensor("ident", [128, 128], mybir.dt.bfloat16,
                        kind="ExternalInput")
    IDTR = nc.dram_tensor("identr", [128, 128], F32R, kind="ExternalInput")
    OUT = nc.dram_tensor("out", [BPC, TFRAMES, NMEL], F32,
                         kind="ExternalOutput")

    def wave_ap(offset, dims):
        return bass.AP(tensor=WAVE, offset=offset, ap=list(dims))

    def out_ap(offset, dims):
        return bass.AP(tensor=OUT, offset=offset, ap=list(dims))

    with tile.TileContext(nc) as tc:
        with tc.tile_pool(name="const", bufs=1) as constp, \
             tc.tile_pool(name="ld", bufs=12) as ldp, \
             tc.tile_pool(name="mv", bufs=2) as mvp, \
             tc.tile_pool(name="sq", bufs=14) as sqp, \
             tc.tile_pool(name="epi", bufs=2) as epp, \
             tc.tile_pool(name="tr_ps", bufs=2, space="PSUM") as trps, \
             tc.tile_pool(name="dft_ps", bufs=2, space="PSUM") as dftps, \
             tc.tile_pool(name="mel_ps", bufs=2, space="PSUM") as melps, \
             tc.tile_pool(name="otr_ps", bufs=1, space="PSUM") as otrps:

            identb = constp.tile([128, 128], mybir.dt.bfloat16)
            nc.sync.dma_start(out=identb[:], in_=IDT.ap())
            identr = constp.tile([128, 128], F32R)
            nc.scalar.dma_start(out=identr[:], in_=IDTR.ap())

            # first block's raw-frame loads go out first on the sync queue;
            # the big DFT-matrix load streams per-chunk on scalar/vector
            # queues so early transposes and chunk-0 matmuls aren't blocked.
            ld_pending = {}

            def issue_ld(b0, nw):
                # frame-major 128-row tiles across wave boundaries: fewer,
                # fuller PE transposes (4/chunk instead of 5)
                nf = nw * NFRAMES
                tiles = []
                for g0 in range(0, nf, 128):
                    rows = min(128, nf - g0)
                    lt = ldp.tile([128, NCHUNK * 128], mybir.dt.bfloat16,
                                  tag="ld")
                    r = 0
                    while r < rows:
                        wb, t0 = divmod(g0 + r, NFRAMES)
                        cnt = min(rows - r, NFRAMES - t0)
                        nc.sync.dma_start(
                            out=lt[r:r + cnt, :],
                            in_=wave_ap((b0 + wb) * SR + t0 * HOP,
                                        [[HOP, cnt], [1, NCHUNK * 128]]),
                        )
                        r += cnt
                    tiles.append((lt, rows))
                ld_pending[b0] = tiles

            blocks = _blocks()
            issue_ld(*blocks[0])

            dftm = constp.tile([128, NCHUNK, 2 * KBINS], mybir.dt.float8e4)
            for half in range(2):
                for c in range(NCHUNK):
                    eng = nc.scalar
                    eng.dma_start(
                        out=dftm[:, c, half * KBINS:(half + 1) * KBINS],
                        in_=DFT.ap()[:, c, half * KBINS:(half + 1) * KBINS],
                    )
            fbd = constp.tile([128, 8, 128], mybir.dt.bfloat16)
            nc.scalar.dma_start(out=fbd[:], in_=FBD.ap())
            PADG = 8  # waveforms per pad DMA
            padt = constp.tile([TFRAMES - NFRAMES, PADG, NMEL], F32)
            nc.vector.memset(padt[:], PADV)
            for bi, (b0, nw) in enumerate(blocks):
                nf = nw * NFRAMES
                ld = ld_pending.pop(b0)
                if bi + 1 < len(blocks):
                    issue_ld(*blocks[bi + 1])

                # transpose to (samples on partitions, frames on free),
                # casting to fp8 for the DoubleRow DFT matmuls
                mv = mvp.tile([128, NCHUNK, NFPAD], mybir.dt.float8e4,
                              tag="mv")
                for c in range(NCHUNK):
                    trp = trps.tile([128, nf], mybir.dt.bfloat16,
                                    tag="trp")
                    for j, (lt, rows) in enumerate(ld):
                        nc.tensor.transpose(
                            trp[:, j * 128:j * 128 + rows],
                            lt[:rows, c * 128:(c + 1) * 128],
                            identb[:rows, :rows],
                        )
                    nc.vector.tensor_copy(mv[:, c, :nf], trp[:])

                # DFT (cos/sin folded with preprocessing), power spectrum
                pw = []
                for kk in range(8):
                    halves = []
                    for half in range(2):
                        base = half * KBINS + kk * 128
                        ps = dftps.tile([128, nf], F32, tag="dftps")
                        for c2 in range(NCHUNK // 2):
                            nc.tensor.matmul(
                                ps[:],
                                dftm[:, 2 * c2:2 * c2 + 2, base:base + 128],
                                mv[:, 2 * c2:2 * c2 + 2, :nf],
                                start=(c2 == 0), stop=(c2 == NCHUNK // 2 - 1),
                                perf_mode=mybir.MatmulPerfMode.DoubleRow,
                            )
                        st = sqp.tile([128, nf], mybir.dt.bfloat16, tag="sq")
                        nc.scalar.square(st[:], ps[:])
                        halves.append(st)
                    pt = sqp.tile([128, nf], mybir.dt.bfloat16, tag="pw")
                    nc.vector.tensor_add(pt[:], halves[0][:], halves[1][:])
                    pw.append(pt)

                # mel: contract Re^2+Im^2 (8 chunks of 128 bins)
                mel = melps.tile([128, nf], F32, tag="mel")
                for kk in range(8):
                    nc.tensor.matmul(mel[:], fbd[:, kk, :], pw[kk][:],
                                     start=(kk == 0), stop=(kk == 7))

                # log-mel + normalize: (ln(max(mel', eps')))/(2*std)
                ot = epp.tile([128, nf], F32R, tag="ot")
                nc.vector.tensor_scalar_max(ot[:], mel[:], EPS_S)
                nc.scalar.activation(ot[:], ot[:],
                                     mybir.ActivationFunctionType.Ln)
                nc.vector.tensor_scalar_mul(ot[:], ot[:], OUT_SCALE)

                # transpose back to (frames on partitions, mel on free)
                otr = otrps.tile([NFRAMES, nw * 128], F32R, tag="otr")
                for wb in range(nw):
                    nc.tensor.transpose(
                        otr[:, wb * 128:(wb + 1) * 128],
                        ot[:, wb * NFRAMES:(wb + 1) * NFRAMES],
                        identr[:],
                    )
                oc = epp.tile([NFRAMES, nw, NMEL], F32, tag="oc")
                nc.vector.tensor_copy(oc[:], otr[:].rearrange(
                    "p (w m) -> p w m", w=nw))
                nc.scalar.dma_start(
                    out=out_ap(b0 * TFRAMES * NMEL,
                               [[NMEL, NFRAMES],
                                [TFRAMES * NMEL, nw],
                                [1, NMEL]]),
                    in_=oc[:],
                )

            # constant pad rows (frames 98..127) for every waveform
            for g0 in range(0, BPC, PADG):
                nc.scalar.dma_start(
                    out=out_ap(g0 * TFRAMES * NMEL + NFRAMES * NMEL,
                               [[NMEL, TFRAMES - NFRAMES],
                                [TFRAMES * NMEL, PADG],
                                [1, NMEL]]),
                    in_=padt[:],
                )

    nc.compile()
    return nc


@functools.lru_cache(maxsize=1)
def _host_constants():
    return _build_host_constants()


def make_in_maps(waveform):
    import ml_dtypes
    waveform = np.ascontiguousarray(np.asarray(waveform, dtype=np.float32))
    assert waveform.shape == (B, SR), waveform.shape
    dftm, fbd = _host_constants()
    shards = waveform.reshape(NCORES, BPC, SR).astype(ml_dtypes.bfloat16)
    ident = np.eye(128, dtype=ml_dtypes.bfloat16)
    identr = np.eye(128, dtype=np.float32)
    return [
        {"wave": np.ascontiguousarray(shards[c]), "dftm": dftm, "fbd": fbd,
         "ident": ident, "identr": identr}
        for c in range(NCORES)
    ]


def kernel(waveform):
    nc = _build_nc()
    in_maps = make_in_maps(waveform)
    res = bass_utils.run_bass_kernel_spmd(
        nc, in_maps, core_ids=list(range(NCORES)), trace=False
    )
    return np.concatenate([res.results[c]["out"] for c in range(NCORES)], axis=0)

